# revision 1
# baseline (speedup 1.0000x reference)
"""Trainium2 Bass kernel for nn_CrossLayer (4-layer cross network + BatchNorm).

Math per layer (reference):
    s   = out @ w_l            # [B] per-row dot
    out = x0 * s[:,None] + b_l + out
    out = (out - mean_B) * rsqrt(var_B + eps)   # BatchNorm1d, no affine

Key observation: BatchNorm (no affine) immediately follows the per-column
constant add of b_l, so b_l shifts only the column mean, which BN removes.
b is therefore mathematically irrelevant and is dropped entirely.

Strategy: data-parallel over the batch across 8 NeuronCores (1024 rows each).
On-chip layout is transposed ("layout B"): features D=2048 on partitions
(16 chunks of 128), batch on the free axis. Everything stays resident in SBUF
for all 4 layers:
  - per-row dot s: TensorE matmuls with a *replicated* stationary matrix
    w_rep[k,m] = w[k] for all m, so the accumulated PSUM result is s already
    broadcast across all 128 partitions.
  - update u = x0*s + out: VectorE tensor_tensor mult + tensor_tensor_reduce
    add (the reduce gives sum(u) for free).
  - sumsq: ScalarE Square activation with accum_out.
  - batch stats need a 16KB AllReduce (sum, sumsq packed [128,32]) per layer.
  - normalize: ScalarE activation Identity with per-partition scale/bias.
"""

import sys

for _p in ("/opt/trn_rl_repo",):
    if _p not in sys.path:
        sys.path.insert(0, _p)

import numpy as np

from concourse import bacc, bass, mybir, tile
from concourse import bass_utils

N_CORES = 8
B, D, L = 8192, 2048, 4
B_LOC = B // N_CORES          # 1024 rows per core
P = 128                       # partitions
NCH = D // P                  # 16 feature chunks
FREE = B_LOC                  # 1024 free elements (batch) per chunk
HALF = 512                    # fp32 matmul moving-N limit (one PSUM bank)
EPS = 1e-5
F32 = mybir.dt.float32
F32R = mybir.dt.float32r
BF16 = mybir.dt.bfloat16
N_SUM_ON_S = 4                # chunks whose batch-sum runs on ScalarE

_CACHE = {}


def _build(singleton_cc=False, stage=99):
    # stage: 1=dot only, 2=+update, 3=+square, 4=+allreduce, 99=full
    nc = bacc.Bacc(
        "TRN2", target_bir_lowering=False, debug=False, num_devices=N_CORES
    )
    xt_in = nc.dram_tensor("xt", [D, B_LOC], BF16, kind="ExternalInput")
    wc_in = nc.dram_tensor("wc", [P, L * NCH], F32, kind="ExternalInput")
    yt_out = nc.dram_tensor("yt", [D, B_LOC], F32, kind="ExternalOutput")

    if singleton_cc:
        AR_GROUPS = [[i] for i in range(N_CORES)]
    else:
        AR_GROUPS = [list(range(N_CORES))]

    with tile.TileContext(nc) as tc:
        with (
            tc.tile_pool(name="big", bufs=1) as big,
            tc.tile_pool(name="wrep", bufs=1) as wrep_pool,
            tc.tile_pool(name="s1p", bufs=2) as s1p,
            tc.tile_pool(name="tsc", bufs=3) as tsc,
            tc.tile_pool(name="stat", bufs=2) as statp,
            tc.tile_pool(name="stat2", bufs=2) as statp2,
            tc.tile_pool(name="ps", bufs=2, space="PSUM") as ps,
            tc.tile_pool(name="sqp", bufs=1, space="PSUM") as sqp,
            tc.tile_pool(name="warmp", bufs=1, space="PSUM") as warmp,
            tc.tile_pool(name="dram", bufs=1, space="DRAM") as dramp,
        ):
            X0 = big.tile([P, NCH, FREE], BF16, tag="x0")
            OUT = big.tile([P, NCH, FREE], F32, tag="out")
            wc = wrep_pool.tile([P, L * NCH], F32, tag="wc")
            ones = wrep_pool.tile([P, P], F32, tag="ones")
            w_rep = wrep_pool.tile([P, (L - 1) * NCH, P], F32, tag="wrep")
            w_rep0 = wrep_pool.tile([P, NCH, P], BF16, tag="wrep0")
            eps_t = wrep_pool.tile([P, 1], F32, tag="eps")
            nc.vector.memset(eps_t[:], EPS)

            # ---- warm up the collectives path (absorbs the ~38us comm-init
            # barrier while input DMA + layer-0 matmuls run) ----
            warm_in = dramp.tile([P, 1], F32, tag="warm_in")
            warm_out = dramp.tile([P, 1], F32, tag="warm_out")
            warm_sb = statp2.tile([P, 1], F32, tag="warm_sb")
            nc.vector.memset(warm_sb[:], 0.0)
            nc.gpsimd.dma_start(warm_in[:], warm_sb[:])
            nc.gpsimd.collective_compute(
                "AllReduce",
                mybir.AluOpType.add,
                replica_groups=AR_GROUPS,
                ins=[warm_in[:].opt()],
                outs=[warm_out[:].opt()],
            )

            # ---- load inputs ----
            # X0 is rounded to f32r in place right after each chunk's DMA:
            # the f32r matmuls require producers tagged as f32r, and the
            # rounding error (~1e-5 rel) is negligible.
            for c in range(NCH):
                nc.sync.dma_start(X0[:, c, :], xt_in[c * P : (c + 1) * P, :])
            nc.sync.dma_start(wc[:], wc_in[:])
            nc.vector.memset(ones[:], 1.0)
            # replicated stationary mats: w_rep[:, j, m] = wc[:, j] for all m
            # (emit layer 0's first so layer 0 matmuls can start early)
            for j in range(NCH):
                nc.vector.tensor_scalar(
                    w_rep0[:, j, :], ones[:], wc[:, j : j + 1], None,
                    mybir.AluOpType.mult,
                )
            for j in range((L - 1) * NCH):
                nc.vector.tensor_scalar(
                    w_rep[:, j, :].bitcast(F32R), ones[:], wc[:, NCH + j : NCH + j + 1],
                    None, mybir.AluOpType.mult,
                )

            for layer in range(L):
                src = X0 if layer == 0 else OUT
                # ---- per-row dot, broadcast across partitions ----
                # psum_s[p, r] = sum_d w[layer, d] * out[d, r]  (same for all p)
                # float32r: single-pass full-rate fp32 matmul (vs LOW_HIGH 2x)
                # h outer: half 0 of s completes early so VectorE can start
                # while TensorE runs half 1.
                psum_s = ps.tile([P, FREE], F32, tag="psum_s")
                for h in range(FREE // HALF):
                    for c in range(NCH):
                        if layer == 0:
                            lhsT = w_rep0[:, c, :]
                            rhs = src[:, c, h * HALF : (h + 1) * HALF]
                        else:
                            lhsT = w_rep[:, (layer - 1) * NCH + c, :].bitcast(F32R)
                            rhs = src[:, c, h * HALF : (h + 1) * HALF].bitcast(F32R)
                        nc.tensor.matmul(
                            psum_s[:, h * HALF : (h + 1) * HALF],
                            lhsT, rhs,
                            start=(c == 0),
                            stop=(c == NCH - 1),
                        )

                # layer-0 update reads X0 (already f32r-rounded); later
                # layers read OUT whose producer (the normalize below) wrote
                # through an f32r view, satisfying the bir verifier.

                HN = NCH // 2
                statsA = statp.tile([P, NCH], F32, tag="statsA")
                statsB = statp.tile([P, NCH], F32, tag="statsB")
                warm_ps = warmp.tile([P, 64], F32, tag="warm_ps")

                def _sum_slot(c):
                    tl = statsA if c < HN else statsB
                    return tl[:, (c % HN) : (c % HN) + 1]

                def _ssq_slot(c):
                    tl = statsA if c < HN else statsB
                    return tl[:, HN + (c % HN) : HN + (c % HN) + 1]

                if stage <= 1:
                    for c in range(NCH):
                        nc.vector.tensor_copy(
                            OUT[:, c, :].bitcast(F32R), psum_s[:]
                        )
                        if layer == L - 1:
                            nc.sync.dma_start(
                                yt_out[c * P : (c + 1) * P, :], OUT[:, c, :]
                            )
                    continue

                # copy s to SBUF in bf16 (2x DVE mode for the mult);
                # layer 0 folds the +1 from u = x0*(s+1)
                s1 = s1p.tile([P, FREE], BF16, tag="s1")
                nc.vector.tensor_scalar(
                    s1[:], psum_s[:], 1.0 if layer == 0 else 0.0, None,
                    mybir.AluOpType.add,
                )
                sum_on_s = set(range(0, NCH, NCH // N_SUM_ON_S)) if N_SUM_ON_S else set()
                for c in range(NCH):
                    if layer == 0:
                        nc.vector.tensor_tensor(
                            OUT[:, c, :].bitcast(F32R),
                            X0[:, c, :], s1[:],
                            mybir.AluOpType.mult,
                        )
                    else:
                        t = tsc.tile([P, FREE], BF16, tag="t")
                        for h in range(FREE // HALF):
                            nc.vector.tensor_tensor(
                                t[:, h * HALF : (h + 1) * HALF],
                                X0[:, c, h * HALF : (h + 1) * HALF],
                                s1[:, h * HALF : (h + 1) * HALF],
                                mybir.AluOpType.mult,
                            )
                        nc.vector.tensor_tensor(
                            OUT[:, c, :].bitcast(F32R), t[:],
                            OUT[:, c, :],
                            mybir.AluOpType.add,
                        )
                    if c % 4 == 2:
                        # dummy matmul spread through the V phase keeps the
                        # PE clock-gate warm for the next layer's dot
                        nc.tensor.matmul(
                            warm_ps[:], w_rep0[:, 0, :],
                            X0[:, c, :64],
                            start=True, stop=True,
                        )
                    if c in sum_on_s:
                        sqd = sqp.tile([P, FREE], F32, tag="sq")
                        nc.scalar.activation(
                            sqd[:], OUT[:, c, :],
                            mybir.ActivationFunctionType.Identity,
                            accum_out=_sum_slot(c),
                        )
                    else:
                        nc.vector.tensor_reduce(
                            _sum_slot(c), OUT[:, c, :],
                            mybir.AxisListType.X, mybir.AluOpType.add,
                        )

                if stage <= 2:
                    if layer == L - 1:
                        for c in range(NCH):
                            nc.sync.dma_start(
                                yt_out[c * P : (c + 1) * P, :], OUT[:, c, :]
                            )
                    continue

                # ---- sum of squares on ScalarE ----
                for c in range(NCH):
                    sq = sqp.tile([P, FREE], F32, tag="sq")
                    nc.scalar.activation(
                        sq[:], OUT[:, c, :],
                        mybir.ActivationFunctionType.Square,
                        accum_out=_ssq_slot(c),
                    )

                if stage <= 3:
                    if layer == L - 1:
                        for c in range(NCH):
                            nc.sync.dma_start(
                                yt_out[c * P : (c + 1) * P, :], OUT[:, c, :]
                            )
                    continue

                # ---- two AllReduces: half A (chunks 0-7) fires while the
                # update of chunks 8-15 is still running, hiding its latency.
                # NB: bounce buffers must be unique per collective.
                invs, nbs = [], []
                for half, stl in ((0, statsA), (1, statsB)):
                    ar_in = dramp.tile([P, NCH], F32, tag=f"ar_in{layer}_{half}")
                    ar_out = dramp.tile([P, NCH], F32, tag=f"ar_out{layer}_{half}")
                    nc.sync.dma_start(ar_in[:], stl[:])
                    nc.gpsimd.collective_compute(
                        "AllReduce",
                        mybir.AluOpType.add,
                        replica_groups=AR_GROUPS,
                        ins=[ar_in[:].opt()],
                        outs=[ar_out[:].opt()],
                    )
                    g = statp.tile([P, NCH], F32, tag=f"gstats{half}")
                    nc.sync.dma_start(g[:], ar_out[:])

                    mu = statp2.tile([P, HN], F32, tag=f"mu{half}")
                    ex2 = statp2.tile([P, HN], F32, tag=f"ex2{half}")
                    var = statp2.tile([P, HN], F32, tag=f"var{half}")
                    sd = statp2.tile([P, HN], F32, tag=f"sd{half}")
                    inv = statp2.tile([P, HN], F32, tag=f"inv{half}")
                    nb = statp2.tile([P, HN], F32, tag=f"nb{half}")
                    nc.vector.tensor_scalar(
                        mu[:], g[:, :HN], 1.0 / B, None, mybir.AluOpType.mult
                    )
                    nc.vector.tensor_scalar(
                        ex2[:], g[:, HN:], 1.0 / B, None, mybir.AluOpType.mult
                    )
                    nc.vector.tensor_tensor(
                        var[:], mu[:], mu[:], mybir.AluOpType.mult
                    )
                    nc.vector.tensor_tensor(
                        var[:], ex2[:], var[:], mybir.AluOpType.subtract
                    )
                    nc.scalar.activation(
                        sd[:], var[:], mybir.ActivationFunctionType.Sqrt,
                        bias=eps_t[:],
                    )
                    nc.vector.reciprocal(inv[:], sd[:])
                    nc.vector.tensor_tensor(
                        nb[:], mu[:], inv[:], mybir.AluOpType.mult
                    )
                    nc.vector.tensor_scalar(
                        nb[:], nb[:], -1.0, None, mybir.AluOpType.mult
                    )
                    invs.append(inv)
                    nbs.append(nb)

                # ---- normalize, split across ScalarE and VectorE ----
                # writes through an f32r view so next layer's matmul accepts it
                for c in range(NCH):
                    inv_h = invs[0] if c < HN else invs[1]
                    nb_h = nbs[0] if c < HN else nbs[1]
                    cc = c % HN
                    if c % 2 == 0:
                        nc.scalar.activation(
                            OUT[:, c, :].bitcast(F32R), OUT[:, c, :],
                            mybir.ActivationFunctionType.Identity,
                            bias=nb_h[:, cc : cc + 1],
                            scale=inv_h[:, cc : cc + 1],
                        )
                    else:
                        nc.vector.tensor_scalar(
                            OUT[:, c, :].bitcast(F32R), OUT[:, c, :],
                            inv_h[:, cc : cc + 1], nb_h[:, cc : cc + 1],
                            mybir.AluOpType.mult, mybir.AluOpType.add,
                        )
                    if layer == L - 1:
                        nc.sync.dma_start(
                            yt_out[c * P : (c + 1) * P, :], OUT[:, c, :]
                        )

    nc.compile()
    return nc


def _get_nc():
    if "nc" not in _CACHE:
        _CACHE["nc"] = _build()
    return _CACHE["nc"]


def kernel(x, w, b=None, **_ignored):
    x = np.ascontiguousarray(np.asarray(x, dtype=np.float32))
    w = np.asarray(w, dtype=np.float32)
    assert x.shape == (B, D) and w.shape == (L, D)

    # w_cols[p, i*NCH + c] = w[i, c*128 + p]
    w_cols = np.ascontiguousarray(
        w.reshape(L, NCH, P).transpose(2, 0, 1).reshape(P, L * NCH)
    )

    import ml_dtypes

    in_maps = []
    for m in range(N_CORES):
        xt = np.ascontiguousarray(
            x[m * B_LOC : (m + 1) * B_LOC, :].T.astype(ml_dtypes.bfloat16)
        )
        in_maps.append({"xt": xt, "wc": w_cols})

    nc = _get_nc()
    res = bass_utils.run_bass_kernel_spmd(
        nc, in_maps, core_ids=list(range(N_CORES))
    )

    out = np.empty((B, D), dtype=np.float32)
    for m in range(N_CORES):
        yt = res.results[m]["yt"]
        out[m * B_LOC : (m + 1) * B_LOC, :] = yt.T
    return out


if __name__ == "__main__":
    xs = np.random.randn(B, D).astype(np.float32)
    ws = np.random.randn(L, D).astype(np.float32)
    bs = np.random.randn(L, D).astype(np.float32)
    y = kernel(xs, ws, bs)
    print("kernel ran, out shape", y.shape)



# revision 16
# speedup vs baseline: 1.3034x; 1.3034x over previous
"""Trainium2 Bass kernel for nn_CrossLayer (4-layer cross network + BatchNorm).

Math per layer (reference):
    s   = out @ w_l            # [B] per-row dot
    out = x0 * s[:,None] + b_l + out
    out = (out - mean_B) * rsqrt(var_B + eps)   # BatchNorm1d, no affine

Algebraic restructuring used here:
  1. b_l only shifts column means, which BN removes -> dropped (baseline).
  2. BN normalization is FOLDED into the next layer instead of materialized:
     keep the unnormalized tensor u with stats (mu, inv = rsqrt(var+eps)).
       s'      = sum_d u[:,d]*(w[d]*inv[d]) - c0,  c0 = sum_d mu*inv*w
       u_next  = x0*s' + u*inv        (the dropped -mu*inv term and +b are
                                       per-column constants, which the next
                                       BN/cc0 pair removes exactly)
     Only the final layer does an explicit normalize for the output.
  3. The batch-sum of u_next comes FREE from scalar_tensor_tensor's
     accum_out during the update; only sum-of-squares needs extra passes
     (ScalarE Square+accum, a few chunks on VectorE tensor_tensor_reduce).
  4. One 16KB AllReduce per layer (sum+sumsq packed [128,32] f32).

Data-parallel over batch across 8 cores (1024 rows each), transposed
layout: features D=2048 on partitions (16 chunks of 128), batch on the
free axis. All state bf16 (2x DVE rate, 2x matmul stream rate); PSUM and
stats accumulate f32.
"""

import sys

for _p in ("/opt/trn_rl_repo",):
    if _p not in sys.path:
        sys.path.insert(0, _p)

import numpy as np

from concourse import bacc, bass, mybir, tile
from concourse import bass_utils

N_CORES = 8
B, D, L = 8192, 2048, 4
B_LOC = B // N_CORES          # 1024 rows per core
P = 128                       # partitions
NCH = D // P                  # 16 feature chunks
FREE = B_LOC                  # 1024 free elements (batch) per chunk
HALF = 512                    # fp32 PSUM bank / matmul moving-N limit
EPS = 1e-5
F32 = mybir.dt.float32
BF16 = mybir.dt.bfloat16
N_SSQ_ON_V = 4                # chunks whose sum-of-squares runs on VectorE

_CACHE = {}


def _build(stage=99):
    # stage bisection: 1 = layer0 mm+s1+update only; 2 = +ssq (21: ScalarE
    # only, 22: VectorE TTR only); 3 = +AR/stats; 99 = all 4 layers
    ssq_mode = {21: "s", 22: "v"}.get(stage, "mix")
    if stage in (21, 22):
        stage = 2
    nc = bacc.Bacc(
        "TRN2", target_bir_lowering=False, debug=False, num_devices=N_CORES
    )
    xt_in = nc.dram_tensor("xt", [D, B_LOC], BF16, kind="ExternalInput")
    w0rep_in = nc.dram_tensor("w0rep", [P, NCH, P], BF16, kind="ExternalInput")
    wc_in = nc.dram_tensor("wc", [P, L * NCH], F32, kind="ExternalInput")
    yt_out = nc.dram_tensor("yt", [D, B_LOC], F32, kind="ExternalOutput")

    AR_GROUPS = [list(range(N_CORES))]
    AOP = mybir.AluOpType
    AF = mybir.ActivationFunctionType

    with tile.TileContext(nc) as tc:
        with (
            tc.tile_pool(name="big", bufs=1) as big,
            tc.tile_pool(name="wpool", bufs=1) as wpool,
            tc.tile_pool(name="s1p", bufs=2) as s1p,
            tc.tile_pool(name="tsc", bufs=3) as tsc,
            tc.tile_pool(name="stat", bufs=2) as statp,
            tc.tile_pool(name="stat2", bufs=2) as statp2,
            tc.tile_pool(name="stage", bufs=3) as stagep,
            tc.tile_pool(name="ps", bufs=2, space="PSUM") as ps,
            tc.tile_pool(name="sqp", bufs=1, space="PSUM") as sqp,
            tc.tile_pool(name="c0p", bufs=1, space="PSUM") as c0p,
            tc.tile_pool(name="warmp", bufs=1, space="PSUM") as warmp,
            tc.tile_pool(name="dram", bufs=1, space="DRAM") as dramp,
        ):
            X0 = big.tile([P, NCH, FREE], BF16, tag="x0")
            U = big.tile([P, NCH, FREE], BF16, tag="u")
            T = big.tile([P, NCH, FREE], BF16, tag="t")
            w0rep = wpool.tile([P, NCH, P], BF16, tag="w0rep")
            wsrep = wpool.tile([P, NCH, P], BF16, tag="wsrep")
            wc = wpool.tile([P, L * NCH], F32, tag="wc")
            ones_bf = wpool.tile([P, P], BF16, tag="ones_bf")
            eps_t = wpool.tile([P, 1], F32, tag="eps")
            nc.vector.memset(eps_t[:], EPS)
            nc.vector.memset(ones_bf[:], 1.0)

            # ---- warm up the collectives path (absorbs the comm-init
            # barrier while input DMA + layer-0 matmuls run) ----
            warm_in = dramp.tile([P, 1], F32, tag="warm_in")
            warm_out = dramp.tile([P, 1], F32, tag="warm_out")
            warm_sb = statp2.tile([P, 1], F32, tag="warm_sb")
            nc.vector.memset(warm_sb[:], 0.0)
            nc.gpsimd.dma_start(warm_in[:], warm_sb[:])
            nc.gpsimd.collective_compute(
                "AllReduce",
                mybir.AluOpType.add,
                replica_groups=AR_GROUPS,
                ins=[warm_in[:].opt()],
                outs=[warm_out[:].opt()],
            )

            # ---- load inputs ----
            nc.sync.dma_start(wc[:], wc_in[:])
            nc.sync.dma_start(w0rep[:], w0rep_in[:])
            for c in range(NCH):
                nc.sync.dma_start(X0[:, c, :], xt_in[c * P : (c + 1) * P, :])

            # per-layer normalization state (from the previous layer's AR)
            inv_t = None   # [P, NCH] f32: rsqrt(var+eps) per feature
            mu_t = None    # [P, NCH] f32
            c0neg = None   # [P, 1] f32: -sum_d mu*inv*w_l (bcast on parts)

            n_layers = 1 if stage < 4 else L
            for layer in range(n_layers):

                if layer > 0:
                    # scaled replicated stationary weights: w_l * inv
                    ws = statp2.tile([P, NCH], F32, tag="ws")
                    wl = wc[:, layer * NCH : (layer + 1) * NCH]
                    nc.vector.tensor_tensor(ws[:], wl, inv_t[:], AOP.mult)
                    for c in range(NCH):
                        if c % 2 == 0:
                            nc.vector.tensor_scalar(
                                wsrep[:, c, :], ones_bf[:], ws[:, c : c + 1],
                                None, AOP.mult,
                            )
                        else:
                            nc.scalar.mul(
                                wsrep[:, c, :], ones_bf[:], ws[:, c : c + 1]
                            )
                    # c0 = sum_d mu*inv*w_l  (cross-partition total via
                    # ones-matmul broadcast)
                    m2 = statp2.tile([P, NCH], F32, tag="m2")
                    nc.vector.tensor_tensor(m2[:], ws[:], mu_t[:], AOP.mult)
                    r1 = statp2.tile([P, 1], F32, tag="r1")
                    nc.vector.tensor_reduce(
                        r1[:], m2[:], mybir.AxisListType.X, AOP.add
                    )
                    r1b = statp2.tile([P, 1], BF16, tag="r1b")
                    nc.vector.tensor_copy(r1b[:], r1[:])
                    c0ps = c0p.tile([P, 1], F32, tag="c0ps")
                    nc.tensor.matmul(
                        c0ps[:], ones_bf[:], r1b[:], start=True, stop=True
                    )
                    c0neg = statp2.tile([P, 1], F32, tag="c0neg")
                    nc.vector.tensor_scalar(
                        c0neg[:], c0ps[:], -1.0, None, AOP.mult
                    )

                # ---- per-row dot, broadcast across partitions ----
                src = X0 if layer == 0 else U
                lrep = w0rep if layer == 0 else wsrep
                psum_s = ps.tile([P, FREE], F32, tag="psum_s")
                s1 = s1p.tile([P, FREE], BF16, tag="s1")
                for h in range(FREE // HALF):
                    hs = slice(h * HALF, (h + 1) * HALF)
                    for c in range(NCH):
                        nc.tensor.matmul(
                            psum_s[:, hs],
                            lrep[:, c, :],
                            src[:, c, hs],
                            start=(c == 0),
                            stop=(c == NCH - 1),
                        )
                    # s1 = s_raw + 1 (layer 0) / s_raw - c0 (later layers);
                    # on ScalarE so VectorE can start the update immediately
                    if layer == 0:
                        nc.scalar.activation(
                            s1[:, hs], psum_s[:, hs], AF.Identity, bias=1.0
                        )
                    else:
                        nc.scalar.activation(
                            s1[:, hs], psum_s[:, hs], AF.Identity,
                            bias=c0neg[:, 0:1],
                        )

                # ---- update u + batch stats ----
                stats = statp.tile([P, 2 * NCH], F32, tag="stats")
                warm_ps = warmp.tile([P, 64], F32, tag="warm_ps")

                if layer > 0:
                    # x0*s1 products for half 0 first: they only need s1's
                    # first half, so they overlap the half-1 matmuls
                    for c in range(NCH):
                        nc.vector.tensor_tensor(
                            T[:, c, 0:HALF], X0[:, c, 0:HALF], s1[:, 0:HALF],
                            AOP.mult,
                        )
                for c in range(NCH):
                    if layer == 0:
                        # u0 = x0 * s1, sum(u0) for free
                        nc.vector.scalar_tensor_tensor(
                            U[:, c, :], X0[:, c, :], 1.0, s1[:],
                            AOP.mult, AOP.mult,
                            accum_out=stats[:, c : c + 1],
                        )
                    else:
                        nc.vector.tensor_tensor(
                            T[:, c, HALF:FREE], X0[:, c, HALF:FREE],
                            s1[:, HALF:FREE], AOP.mult,
                        )
                        # u = u*inv + x0*s1 (in place), sum(u) for free
                        nc.vector.scalar_tensor_tensor(
                            U[:, c, :], U[:, c, :], inv_t[:, c : c + 1],
                            T[:, c, :],
                            AOP.mult, AOP.add,
                            accum_out=stats[:, c : c + 1],
                        )
                    # sum of squares
                    if stage == 1:
                        continue
                    if ssq_mode == "s":
                        use_v = False   # all sumsq on ScalarE
                    elif ssq_mode == "v":
                        use_v = True    # all sumsq on VectorE TTR
                    else:
                        use_v = c % (NCH // N_SSQ_ON_V) == 2
                    if use_v:
                        junk = tsc.tile([P, FREE], BF16, tag="t")
                        nc.vector.scalar_tensor_tensor(
                            junk[:], U[:, c, :], 1.0, U[:, c, :],
                            AOP.mult, AOP.mult,
                            accum_out=stats[:, NCH + c : NCH + c + 1],
                        )
                    else:
                        sq = sqp.tile([P, FREE], F32, tag="sq")
                        nc.scalar.activation(
                            sq[:], U[:, c, :], AF.Square,
                            accum_out=stats[:, NCH + c : NCH + c + 1],
                        )
                    if c % 8 == 4:
                        # dummy matmul keeps the PE clock-gate warm
                        nc.tensor.matmul(
                            warm_ps[:], w0rep[:, 0, :], X0[:, c, :64],
                            start=True, stop=True,
                        )

                if stage <= 2:
                    continue

                # ---- one AllReduce per layer: [sum | sumsq] ----
                ar_in = dramp.tile([P, 2 * NCH], F32, tag=f"ar_in{layer}")
                ar_out = dramp.tile([P, 2 * NCH], F32, tag=f"ar_out{layer}")
                nc.sync.dma_start(ar_in[:], stats[:])
                nc.gpsimd.collective_compute(
                    "AllReduce",
                    mybir.AluOpType.add,
                    replica_groups=AR_GROUPS,
                    ins=[ar_in[:].opt()],
                    outs=[ar_out[:].opt()],
                )
                g = statp.tile([P, 2 * NCH], F32, tag="g")
                nc.sync.dma_start(g[:], ar_out[:])

                # ---- global stats -> mu, inv ----
                mu_t = statp2.tile([P, NCH], F32, tag="mu")
                mu2 = statp2.tile([P, NCH], F32, tag="mu2")
                var = statp2.tile([P, NCH], F32, tag="var")
                sd = statp2.tile([P, NCH], F32, tag="sd")
                inv_t = statp2.tile([P, NCH], F32, tag="inv")
                nc.vector.tensor_scalar(
                    mu_t[:], g[:, :NCH], 1.0 / B, None, AOP.mult
                )
                nc.vector.tensor_tensor(mu2[:], mu_t[:], mu_t[:], AOP.mult)
                nc.vector.scalar_tensor_tensor(
                    var[:], g[:, NCH:], 1.0 / B, mu2[:],
                    AOP.mult, AOP.subtract,
                )
                nc.scalar.activation(
                    sd[:], var[:], AF.Sqrt, bias=eps_t[:]
                )
                nc.vector.reciprocal(inv_t[:], sd[:])

            # ---- final explicit normalize + output ----
            if stage <= 2:
                for c in range(NCH):
                    stg = stagep.tile([P, FREE], F32, tag="stg")
                    nc.vector.tensor_copy(stg[:], U[:, c, :])
                    nc.sync.dma_start(yt_out[c * P : (c + 1) * P, :], stg[:])
            else:
                nb = statp2.tile([P, NCH], F32, tag="nb")
                nc.vector.tensor_tensor(nb[:], mu_t[:], inv_t[:], AOP.mult)
                nc.vector.tensor_scalar(nb[:], nb[:], -1.0, None, AOP.mult)
                for c in range(NCH):
                    stg = stagep.tile([P, FREE], F32, tag="stg")
                    if c % 2 == 0:
                        nc.scalar.activation(
                            stg[:], U[:, c, :], AF.Identity,
                            bias=nb[:, c : c + 1], scale=inv_t[:, c : c + 1],
                        )
                    else:
                        nc.vector.tensor_scalar(
                            stg[:], U[:, c, :], inv_t[:, c : c + 1],
                            nb[:, c : c + 1], AOP.mult, AOP.add,
                        )
                    nc.sync.dma_start(yt_out[c * P : (c + 1) * P, :], stg[:])

    nc.compile()
    return nc


def _get_nc():
    if "nc" not in _CACHE:
        _CACHE["nc"] = _build()
    return _CACHE["nc"]


def _prep_inputs(x, w):
    import ml_dtypes

    # w_cols[p, l*NCH + c] = w[l, c*128 + p]
    w_cols = np.ascontiguousarray(
        w.reshape(L, NCH, P).transpose(2, 0, 1).reshape(P, L * NCH)
    )
    # replicated stationary for layer 0: w0rep[p, c, m] = w[0, c*128+p]
    w0rep = np.ascontiguousarray(
        np.broadcast_to(
            w[0].reshape(NCH, P).T[:, :, None], (P, NCH, P)
        ).astype(ml_dtypes.bfloat16)
    )
    in_maps = []
    for m in range(N_CORES):
        xt = np.ascontiguousarray(
            x[m * B_LOC : (m + 1) * B_LOC, :].T.astype(ml_dtypes.bfloat16)
        )
        in_maps.append({"xt": xt, "wc": w_cols, "w0rep": w0rep})
    return in_maps


def kernel(x, w, b=None, **_ignored):
    x = np.ascontiguousarray(np.asarray(x, dtype=np.float32))
    w = np.asarray(w, dtype=np.float32)
    assert x.shape == (B, D) and w.shape == (L, D)

    in_maps = _prep_inputs(x, w)
    nc = _get_nc()
    res = bass_utils.run_bass_kernel_spmd(
        nc, in_maps, core_ids=list(range(N_CORES))
    )

    out = np.empty((B, D), dtype=np.float32)
    for m in range(N_CORES):
        yt = res.results[m]["yt"]
        out[m * B_LOC : (m + 1) * B_LOC, :] = yt.T
    return out


if __name__ == "__main__":
    xs = np.random.randn(B, D).astype(np.float32)
    ws = np.random.randn(L, D).astype(np.float32)
    bs = np.random.randn(L, D).astype(np.float32)
    y = kernel(xs, ws, bs)
    print("kernel ran, out shape", y.shape)


# revision 23
# speedup vs baseline: 1.3702x; 1.0513x over previous
"""Trainium2 Bass kernel for nn_CrossLayer (4-layer cross network + BatchNorm).

Math per layer (reference):
    s   = out @ w_l            # [B] per-row dot
    out = x0 * s[:,None] + b_l + out
    out = (out - mean_B) * rsqrt(var_B + eps)   # BatchNorm1d, no affine

Algebraic restructuring used here:
  1. b_l only shifts column means, which BN removes -> dropped (baseline).
  2. BN normalization is FOLDED into the next layer instead of materialized:
     keep the unnormalized tensor u with stats (mu, inv = rsqrt(var+eps)).
       s'      = sum_d u[:,d]*(w[d]*inv[d]) - c0,  c0 = sum_d mu*inv*w
       u_next  = x0*s' + u*inv        (the dropped -mu*inv term and +b are
                                       per-column constants, which the next
                                       BN/cc0 pair removes exactly)
     Only the final layer does an explicit normalize for the output.
  3. The batch-sum of u_next comes FREE from scalar_tensor_tensor's
     accum_out during the update; only sum-of-squares needs extra passes
     (ScalarE Square+accum, a few chunks on VectorE tensor_tensor_reduce).
  4. One 16KB AllReduce per layer (sum+sumsq packed [128,32] f32).

Data-parallel over batch across 8 cores (1024 rows each), transposed
layout: features D=2048 on partitions (16 chunks of 128), batch on the
free axis. All state bf16 (2x DVE rate, 2x matmul stream rate); PSUM and
stats accumulate f32.
"""

import sys

for _p in ("/opt/trn_rl_repo",):
    if _p not in sys.path:
        sys.path.insert(0, _p)

import numpy as np

from concourse import bacc, bass, mybir, tile
from concourse import bass_utils

N_CORES = 8
B, D, L = 8192, 2048, 4
B_LOC = B // N_CORES          # 1024 rows per core
P = 128                       # partitions
NCH = D // P                  # 16 feature chunks
FREE = B_LOC                  # 1024 free elements (batch) per chunk
HALF = 512                    # fp32 PSUM bank / matmul moving-N limit
EPS = 1e-5
F32 = mybir.dt.float32
BF16 = mybir.dt.bfloat16
N_SSQ_ON_V = 2                # chunks whose sum-of-squares runs on VectorE
TMUL_ON_G = False             # GpSimd cannot run DVE tensor ops on V3
L0_G_STT = 0                  # layer-0 update chunks handled by GpSimd

_CACHE = {}


def _build(stage=99):
    # stage bisection: 1 = layer0 mm+s1+update only; 2 = +ssq (21: ScalarE
    # only, 22: VectorE TTR only); 3 = +AR/stats; 99 = all 4 layers
    ssq_mode = {21: "s", 22: "v"}.get(stage, "mix")
    if stage in (21, 22):
        stage = 2
    nc = bacc.Bacc(
        "TRN2", target_bir_lowering=False, debug=False, num_devices=N_CORES
    )
    xt_in = nc.dram_tensor("xt", [D, B_LOC], BF16, kind="ExternalInput")
    w0rep_in = nc.dram_tensor("w0rep", [P, NCH, P], BF16, kind="ExternalInput")
    wc_in = nc.dram_tensor("wc", [P, L * NCH], F32, kind="ExternalInput")
    yt_out = nc.dram_tensor("yt", [D, B_LOC], F32, kind="ExternalOutput")

    AR_GROUPS = [list(range(N_CORES))]
    AOP = mybir.AluOpType
    AF = mybir.ActivationFunctionType

    with tile.TileContext(nc) as tc:
        with (
            tc.tile_pool(name="big", bufs=1) as big,
            tc.tile_pool(name="wpool", bufs=1) as wpool,
            tc.tile_pool(name="s1p", bufs=2) as s1p,
            tc.tile_pool(name="tsc", bufs=3) as tsc,
            tc.tile_pool(name="stat", bufs=2) as statp,
            tc.tile_pool(name="stat2", bufs=2) as statp2,
            tc.tile_pool(name="stage", bufs=3) as stagep,
            tc.tile_pool(name="ps", bufs=2, space="PSUM") as ps,
            tc.tile_pool(name="sqp", bufs=1, space="PSUM") as sqp,
            tc.tile_pool(name="c0p", bufs=1, space="PSUM") as c0p,
            tc.tile_pool(name="warmp", bufs=1, space="PSUM") as warmp,
            tc.tile_pool(name="dram", bufs=1, space="DRAM") as dramp,
        ):
            X0 = big.tile([P, NCH, FREE], BF16, tag="x0")
            U = big.tile([P, NCH, FREE], BF16, tag="u")
            T = big.tile([P, NCH, FREE], BF16, tag="t")
            w0rep = wpool.tile([P, NCH, P], BF16, tag="w0rep")
            wsrep = wpool.tile([P, NCH, P], BF16, tag="wsrep")
            wc = wpool.tile([P, L * NCH], F32, tag="wc")
            ones_bf = wpool.tile([P, P], BF16, tag="ones_bf")
            eps_t = wpool.tile([P, 1], F32, tag="eps")
            nc.vector.memset(eps_t[:], EPS)
            nc.vector.memset(ones_bf[:], 1.0)

            # No warm-up collective: layer-0 stats are ready well before the
            # CC init barrier (~66us) completes, so the first real AllReduce
            # absorbs the init itself; a warm AR would only serialize ahead
            # of it and cost an extra ~12us.

            # ---- load inputs ----
            nc.sync.dma_start(wc[:], wc_in[:])
            nc.sync.dma_start(w0rep[:], w0rep_in[:])
            for c in range(NCH):
                nc.sync.dma_start(X0[:, c, :], xt_in[c * P : (c + 1) * P, :])

            # per-layer normalization state (from the previous layer's AR)
            inv_t = None   # [P, NCH] f32: rsqrt(var+eps) per feature
            mu_t = None    # [P, NCH] f32
            c0neg = None   # [P, 1] f32: -sum_d mu*inv*w_l (bcast on parts)

            n_layers = 1 if stage < 4 else L
            for layer in range(n_layers):

                if layer > 0:
                    # scaled replicated stationary weights: w_l * inv
                    ws = statp2.tile([P, NCH], F32, tag="ws")
                    wl = wc[:, layer * NCH : (layer + 1) * NCH]
                    nc.vector.tensor_tensor(ws[:], wl, inv_t[:], AOP.mult)
                    for c in range(NCH):
                        if c % 2 == 0:
                            nc.vector.tensor_scalar(
                                wsrep[:, c, :], ones_bf[:], ws[:, c : c + 1],
                                None, AOP.mult,
                            )
                        else:
                            nc.scalar.mul(
                                wsrep[:, c, :], ones_bf[:], ws[:, c : c + 1]
                            )
                    # c0 = sum_d mu*inv*w_l  (cross-partition total via
                    # ones-matmul broadcast)
                    m2 = statp2.tile([P, NCH], F32, tag="m2")
                    nc.vector.tensor_tensor(m2[:], ws[:], mu_t[:], AOP.mult)
                    r1 = statp2.tile([P, 1], F32, tag="r1")
                    nc.vector.tensor_reduce(
                        r1[:], m2[:], mybir.AxisListType.X, AOP.add
                    )
                    r1b = statp2.tile([P, 1], BF16, tag="r1b")
                    nc.vector.tensor_copy(r1b[:], r1[:])
                    c0ps = c0p.tile([P, 1], F32, tag="c0ps")
                    nc.tensor.matmul(
                        c0ps[:], ones_bf[:], r1b[:], start=True, stop=True
                    )
                    c0neg = statp2.tile([P, 1], F32, tag="c0neg")
                    nc.vector.tensor_scalar(
                        c0neg[:], c0ps[:], -1.0, None, AOP.mult
                    )

                # ---- per-row dot, broadcast across partitions ----
                src = X0 if layer == 0 else U
                lrep = w0rep if layer == 0 else wsrep
                psum_s = ps.tile([P, FREE], F32, tag="psum_s")
                s1 = s1p.tile([P, FREE], BF16, tag="s1")
                for h in range(FREE // HALF):
                    hs = slice(h * HALF, (h + 1) * HALF)
                    for c in range(NCH):
                        nc.tensor.matmul(
                            psum_s[:, hs],
                            lrep[:, c, :],
                            src[:, c, hs],
                            start=(c == 0),
                            stop=(c == NCH - 1),
                        )
                    # s1 = s_raw + 1 (layer 0) / s_raw - c0 (later layers);
                    # on ScalarE so VectorE can start the update immediately
                    if layer == 0:
                        nc.scalar.activation(
                            s1[:, hs], psum_s[:, hs], AF.Identity, bias=1.0
                        )
                    else:
                        nc.scalar.activation(
                            s1[:, hs], psum_s[:, hs], AF.Identity,
                            bias=c0neg[:, 0:1],
                        )

                # ---- update u + batch stats ----
                stats = statp.tile([P, 2 * NCH], F32, tag="stats")
                warm_ps = warmp.tile([P, 64], F32, tag="warm_ps")

                tm_eng = nc.gpsimd if TMUL_ON_G else nc.vector
                if layer > 0:
                    # x0*s1 products on GpSimd (otherwise idle), half 0
                    # first so they overlap the half-1 matmuls
                    for c in range(NCH):
                        tm_eng.tensor_tensor(
                            T[:, c, 0:HALF], X0[:, c, 0:HALF], s1[:, 0:HALF],
                            AOP.mult,
                        )
                    for c in range(NCH):
                        tm_eng.tensor_tensor(
                            T[:, c, HALF:FREE], X0[:, c, HALF:FREE],
                            s1[:, HALF:FREE], AOP.mult,
                        )
                for c in range(NCH):
                    if layer == 0:
                        # u0 = x0 * s1, sum(u0) for free
                        eng = nc.gpsimd if c >= NCH - L0_G_STT else nc.vector
                        eng.scalar_tensor_tensor(
                            U[:, c, :], X0[:, c, :], 1.0, s1[:],
                            AOP.mult, AOP.mult,
                            accum_out=stats[:, c : c + 1],
                        )
                    else:
                        # u = u*inv + x0*s1 (in place), sum(u) for free
                        nc.vector.scalar_tensor_tensor(
                            U[:, c, :], U[:, c, :], inv_t[:, c : c + 1],
                            T[:, c, :],
                            AOP.mult, AOP.add,
                            accum_out=stats[:, c : c + 1],
                        )
                    # sum of squares
                    if stage == 1:
                        continue
                    if ssq_mode == "s":
                        use_v = False   # all sumsq on ScalarE
                    elif ssq_mode == "v":
                        use_v = True    # all sumsq on VectorE
                    else:
                        use_v = N_SSQ_ON_V > 0 and c % (NCH // N_SSQ_ON_V) == 2
                    if use_v:
                        junk = tsc.tile([P, FREE], BF16, tag="t")
                        nc.vector.scalar_tensor_tensor(
                            junk[:], U[:, c, :], 1.0, U[:, c, :],
                            AOP.mult, AOP.mult,
                            accum_out=stats[:, NCH + c : NCH + c + 1],
                        )
                    else:
                        sq = sqp.tile([P, FREE], F32, tag="sq")
                        nc.scalar.activation(
                            sq[:], U[:, c, :], AF.Square,
                            accum_out=stats[:, NCH + c : NCH + c + 1],
                        )
                    if c % 8 == 4:
                        # dummy matmul keeps the PE clock-gate warm
                        nc.tensor.matmul(
                            warm_ps[:], w0rep[:, 0, :], X0[:, c, :64],
                            start=True, stop=True,
                        )

                if stage <= 2:
                    continue

                # ---- one AllReduce per layer: [sum | sumsq] ----
                ar_in = dramp.tile([P, 2 * NCH], F32, tag=f"ar_in{layer}")
                ar_out = dramp.tile([P, 2 * NCH], F32, tag=f"ar_out{layer}")
                nc.sync.dma_start(ar_in[:], stats[:])
                nc.gpsimd.collective_compute(
                    "AllReduce",
                    mybir.AluOpType.add,
                    replica_groups=AR_GROUPS,
                    ins=[ar_in[:].opt()],
                    outs=[ar_out[:].opt()],
                )
                g = statp.tile([P, 2 * NCH], F32, tag="g")
                nc.sync.dma_start(g[:], ar_out[:])

                # ---- global stats -> mu, inv ----
                mu_t = statp2.tile([P, NCH], F32, tag="mu")
                mu2 = statp2.tile([P, NCH], F32, tag="mu2")
                var = statp2.tile([P, NCH], F32, tag="var")
                sd = statp2.tile([P, NCH], F32, tag="sd")
                inv_t = statp2.tile([P, NCH], F32, tag="inv")
                nc.vector.tensor_scalar(
                    mu_t[:], g[:, :NCH], 1.0 / B, None, AOP.mult
                )
                nc.vector.tensor_tensor(mu2[:], mu_t[:], mu_t[:], AOP.mult)
                nc.vector.scalar_tensor_tensor(
                    var[:], g[:, NCH:], 1.0 / B, mu2[:],
                    AOP.mult, AOP.subtract,
                )
                nc.scalar.activation(
                    sd[:], var[:], AF.Sqrt, bias=eps_t[:]
                )
                nc.vector.reciprocal(inv_t[:], sd[:])

            # ---- final explicit normalize + output ----
            if stage <= 2:
                for c in range(NCH):
                    stg = stagep.tile([P, FREE], F32, tag="stg")
                    nc.vector.tensor_copy(stg[:], U[:, c, :])
                    nc.sync.dma_start(yt_out[c * P : (c + 1) * P, :], stg[:])
            else:
                nb = statp2.tile([P, NCH], F32, tag="nb")
                nc.vector.tensor_tensor(nb[:], mu_t[:], inv_t[:], AOP.mult)
                nc.vector.tensor_scalar(nb[:], nb[:], -1.0, None, AOP.mult)
                for c in range(NCH):
                    stg = stagep.tile([P, FREE], F32, tag="stg")
                    if c % 2 == 0:
                        nc.scalar.activation(
                            stg[:], U[:, c, :], AF.Identity,
                            bias=nb[:, c : c + 1], scale=inv_t[:, c : c + 1],
                        )
                        nc.sync.dma_start(
                            yt_out[c * P : (c + 1) * P, :], stg[:]
                        )
                    else:
                        nc.vector.tensor_scalar(
                            stg[:], U[:, c, :], inv_t[:, c : c + 1],
                            nb[:, c : c + 1], AOP.mult, AOP.add,
                        )
                        # second DMA queue so the output drain halves
                        nc.gpsimd.dma_start(
                            yt_out[c * P : (c + 1) * P, :], stg[:]
                        )

    nc.compile()
    return nc


def _get_nc():
    if "nc" not in _CACHE:
        _CACHE["nc"] = _build()
    return _CACHE["nc"]


def _prep_inputs(x, w):
    import ml_dtypes

    # w_cols[p, l*NCH + c] = w[l, c*128 + p]
    w_cols = np.ascontiguousarray(
        w.reshape(L, NCH, P).transpose(2, 0, 1).reshape(P, L * NCH)
    )
    # replicated stationary for layer 0: w0rep[p, c, m] = w[0, c*128+p]
    w0rep = np.ascontiguousarray(
        np.broadcast_to(
            w[0].reshape(NCH, P).T[:, :, None], (P, NCH, P)
        ).astype(ml_dtypes.bfloat16)
    )
    in_maps = []
    for m in range(N_CORES):
        xt = np.ascontiguousarray(
            x[m * B_LOC : (m + 1) * B_LOC, :].T.astype(ml_dtypes.bfloat16)
        )
        in_maps.append({"xt": xt, "wc": w_cols, "w0rep": w0rep})
    return in_maps


def kernel(x, w, b=None, **_ignored):
    x = np.ascontiguousarray(np.asarray(x, dtype=np.float32))
    w = np.asarray(w, dtype=np.float32)
    assert x.shape == (B, D) and w.shape == (L, D)

    in_maps = _prep_inputs(x, w)
    nc = _get_nc()
    res = bass_utils.run_bass_kernel_spmd(
        nc, in_maps, core_ids=list(range(N_CORES))
    )

    out = np.empty((B, D), dtype=np.float32)
    for m in range(N_CORES):
        yt = res.results[m]["yt"]
        out[m * B_LOC : (m + 1) * B_LOC, :] = yt.T
    return out


if __name__ == "__main__":
    xs = np.random.randn(B, D).astype(np.float32)
    ws = np.random.randn(L, D).astype(np.float32)
    bs = np.random.randn(L, D).astype(np.float32)
    y = kernel(xs, ws, bs)
    print("kernel ran, out shape", y.shape)
